# revision 22
# baseline (speedup 1.0000x reference)
"""Trainium2 Bass kernel for a DANet-style dual-attention head (v3).

Full inputs in, full outputs out.  4 samples x 2 branches = 8 independent
units, one per NeuronCore.  A single uniform program runs on all 8 cores:

    CBR(w1) -> CAM(g1) -> PAM -> CAM(g2) -> CBR(w2) -> qkv 1x1 partials
    -> pairwise AllReduce of qkv partials -> tiny row-attention -> out

A-branch cores get (g1=cam_gamma, g2=0); B-branch cores get (g1=0,
g2=cam_gamma).  CAM with gamma=0 is exactly the identity.

v3 vs v2:
  - conv2 also tap-pair packed (5 matmuls/slice) via on-device shifted
    copies of the padded CAM2 output (reusing conv1's T66/T1 SBUF slots).
  - CAM applies in bf16 (featB/y2b shadow casts); CAM energies stay fp32.
  - deeper PAM pipeline: po drained to SBUF immediately; normalization
    runs one slice later so the ones-broadcast matmul never stalls the PE.
  - AllReduce split in two halves, first half overlaps conv2's second half.
  - startup DMAs spread across the sync and scalar DMA queues.
"""

from contextlib import ExitStack

import ml_dtypes
import numpy as np

import concourse.bacc as bacc
import concourse.bass as bass
import concourse.tile as tile
from concourse import mybir
from concourse.bass_utils import run_bass_kernel_spmd
from concourse.masks import make_identity

F32 = mybir.dt.float32
BF16 = mybir.dt.bfloat16

B, C, H, W = 4, 64, 64, 64
N = H * W            # 4096
C8 = C // 8          # 8   (pam q/k channels)
CI = C // 2          # 32  (conv51/conv52 out channels)
HP, WP = H + 2, W + 2
PADN = HP * WP       # 4356
SL = 512             # free-dim slice width (8 image rows)
NSL = N // SL        # 8 slices
NCH = N // 128       # 32 chunks of 128 positions
EPS = 1e-5

# PAM energy PSUM groups per n-slice: alternating 3-chunk / 2-chunk groups.
E_GROUPS = [(0, 3), (3, 2), (5, 3), (8, 2), (10, 3), (13, 2), (15, 3), (18, 2),
            (20, 3), (23, 2), (25, 3), (28, 2), (30, 2)]
assert sum(g[1] for g in E_GROUPS) == NCH

REPLICA_GROUPS = [[0, 1], [2, 3], [4, 5], [6, 7]]


def _cam_softmax(nc, misc, psum_t, energy_psum, identity_b):
    """softmax(rowmax(E) - E, axis=-1) on a [64, 64] PSUM tile -> attT bf16."""
    m1 = misc.tile([C, 1], F32, tag="cm1")
    nc.vector.reduce_max(out=m1, in_=energy_psum, axis=mybir.AxisListType.X)
    en = misc.tile([C, C], F32, tag="cen")
    # en = (E - m1) * -1 = rowmax - E
    nc.vector.tensor_scalar(en, energy_psum, m1, -1.0,
                            mybir.AluOpType.subtract, mybir.AluOpType.mult)
    m2 = misc.tile([C, 1], F32, tag="cm2")
    nc.vector.reduce_max(out=m2, in_=en, axis=mybir.AxisListType.X, negate=True)
    ex = misc.tile([C, C], F32, tag="cex")
    ssum = misc.tile([C, 1], F32, tag="css")
    nc.scalar.activation(out=ex, in_=en, func=mybir.ActivationFunctionType.Exp,
                         bias=m2, scale=1.0, accum_out=ssum)
    rr = misc.tile([C, 1], F32, tag="crr")
    nc.vector.reciprocal(out=rr, in_=ssum)
    att = misc.tile([C, C], BF16, tag="catt")
    nc.vector.tensor_scalar_mul(att, ex, rr)
    pt = psum_t.tile([C, C], BF16, tag="tr")
    nc.tensor.transpose(pt, att[:], identity_b[0:C, 0:C])
    attT = misc.tile([C, C], BF16, tag="cattT")
    nc.vector.tensor_copy(out=attT, in_=pt)
    return attT


def build_nc(phases=5):
    nc = bacc.Bacc("TRN2", target_bir_lowering=False, debug=False, num_devices=8)

    # x arrives pre-padded [C, 66*66] bf16 from the host.
    x_in = nc.declare_dram_parameter("x", [C, PADN], BF16, isOutput=False)
    # conv weights pre-packed on host: [128, 5, Cout] bf16 (tap pairs).
    w1p_in = nc.declare_dram_parameter("w1p", [128, 5 * C], BF16, isOutput=False)
    w2p_in = nc.declare_dram_parameter("w2p", [128, 5 * CI], BF16, isOutput=False)
    qkr_in = nc.declare_dram_parameter("qkr", [128, 256], BF16, isOutput=False)
    qkb_in = nc.declare_dram_parameter("qkb", [128, 2], F32, isOutput=False)
    mask_in = nc.declare_dram_parameter("mask", [128, 128], BF16, isOutput=False)
    vwt_in = nc.declare_dram_parameter("vwt", [128, C], BF16, isOutput=False)
    sqkvt_in = nc.declare_dram_parameter("sqkvt", [CI, 3], BF16, isOutput=False)
    vecs_in = nc.declare_dram_parameter("vecs", [C, 8], F32, isOutput=False)
    out_ext = nc.declare_dram_parameter("out", [H, W], F32, isOutput=True)

    with tile.TileContext(nc) as tc, ExitStack() as ctx:
        consts = ctx.enter_context(tc.tile_pool(name="consts", bufs=1))
        pads = ctx.enter_context(tc.tile_pool(name="pads", bufs=1))
        maps = ctx.enter_context(tc.tile_pool(name="maps", bufs=1))
        big = ctx.enter_context(tc.tile_pool(name="big", bufs=1))
        expp = ctx.enter_context(tc.tile_pool(name="expp", bufs=2))
        misc = ctx.enter_context(tc.tile_pool(name="misc", bufs=2))
        dram = ctx.enter_context(tc.tile_pool(name="dram", bufs=1, space="DRAM"))
        # PSUM: psA(2 banks) + psT(1) + peA(3) + peB(2) = 8 banks
        psA = ctx.enter_context(tc.tile_pool(name="psA", bufs=2, space="PSUM"))
        psT = ctx.enter_context(tc.tile_pool(name="psT", bufs=1, space="PSUM"))
        peA = ctx.enter_context(tc.tile_pool(name="peA", bufs=1, space="PSUM"))
        peB = ctx.enter_context(tc.tile_pool(name="peB", bufs=1, space="PSUM"))

        def pe_pool(i):
            # double-buffer small PSUM tiles through the (phase-idle) energy banks
            return (peA, "eA") if i % 2 == 0 else (peB, "eB")

        # ---- constants / weights to SBUF ----
        identity_b = consts.tile([128, 128], BF16)
        make_identity(nc, identity_b)
        identity_f = consts.tile([128, 128], F32)
        make_identity(nc, identity_f)
        w1p = consts.tile([128, 5, C], BF16)
        nc.sync.dma_start(out=w1p, in_=w1p_in[:].rearrange("p (j co) -> p j co", j=5))
        w2p = consts.tile([128, 5, CI], BF16)
        nc.sync.dma_start(out=w2p, in_=w2p_in[:].rearrange("p (j co) -> p j co", j=5))
        qkr = consts.tile([128, 256], BF16)
        nc.scalar.dma_start(out=qkr, in_=qkr_in[:])
        qkb = consts.tile([128, 2], F32)
        nc.scalar.dma_start(out=qkb, in_=qkb_in[:])
        maskb = consts.tile([128, 128], BF16)
        nc.scalar.dma_start(out=maskb, in_=mask_in[:])
        vwt = consts.tile([128, C], BF16)
        nc.scalar.dma_start(out=vwt, in_=vwt_in[:])
        sqkvt = consts.tile([CI, 3], BF16)
        nc.scalar.dma_start(out=sqkvt, in_=sqkvt_in[:])
        vecs = consts.tile([C, 8], F32)
        nc.sync.dma_start(out=vecs, in_=vecs_in[:])
        wdum = consts.tile([128, 512], BF16)
        nc.gpsimd.memset(wdum, 0.25)

        def warm_burst(n):
            # gap-free dummy matmuls: keeps the PE 100%-busy for a full HAM
            # SHORT window so the clock un-throttles to 2.4 GHz while the
            # serial softmax runs on the scalar/vector engines.
            for i in range(n):
                pd = psA.tile([128, SL], F32, tag="a")
                nc.tensor.matmul(pd, wdum[:, 0:128], wdum[:], start=True, stop=True)
        # ---- stacked shifted input copies for conv1 tap-pair matmuls ----
        # T66: rows 0:64 = xpad, rows 64:128 = xpad shifted +66 (dy+1)
        # T1:  rows 0:64 = xpad, rows 64:128 = xpad shifted +1  (dx+1)
        T66 = pads.tile([128, HP + 1, WP], BF16, tag="t66")
        T1 = pads.tile([128, HP + 1, WP], BF16, tag="t1")
        T66f = T66[:].rearrange("p h w -> p (h w)")
        T1f = T1[:].rearrange("p h w -> p (h w)")
        nc.gpsimd.memset(T1[:, HP:HP + 1, :], 0.0)  # guard tail read of tap (2,2)
        CUT = 35 * WP  # rows 0-34 cover conv slices 0-3
        nc.sync.dma_start(out=T66f[0:C, 0:CUT], in_=x_in[:, 0:CUT])
        nc.sync.dma_start(out=T66f[C:128, 0:CUT], in_=x_in[:, WP:WP + CUT])
        nc.scalar.dma_start(out=T1f[0:C, 0:CUT], in_=x_in[:, 0:CUT])
        nc.scalar.dma_start(out=T1f[C:128, 0:CUT], in_=x_in[:, 1:1 + CUT])
        nc.sync.dma_start(out=T66f[0:C, CUT:PADN], in_=x_in[:, CUT:PADN])
        nc.sync.dma_start(out=T66f[C:128, CUT:PADN - WP], in_=x_in[:, WP + CUT:PADN])
        nc.scalar.dma_start(out=T1f[0:C, CUT:PADN], in_=x_in[:, CUT:PADN])
        nc.scalar.dma_start(out=T1f[C:128, CUT:PADN - 1], in_=x_in[:, 1 + CUT:PADN])

        b1v = vecs[:, 0:1]
        g1v = vecs[:, 1:2]
        g2v = vecs[:, 2:3]
        gpv = vecs[:, 3:4]
        gpvbv = vecs[:, 4:5]
        b2v = vecs[0:CI, 5:6]

        feat = maps.tile([C, N], F32, tag="feat")
        featB = maps.tile([C, N], BF16, tag="featB")
        y1 = maps.tile([C, N], F32, tag="y1")
        y1b = maps.tile([128, N], BF16, tag="y1b")
        nc.gpsimd.memset(y1b[C:128, :], 0.0)
        xfT = big.tile([128, NCH, C], F32, tag="xfT")

        def conv_pair_slice(s, wp, cout, Ta, Tb):
            """conv3x3 for 8 output rows: 5 tap-pair matmuls into one PSUM tile."""
            r0 = s * 8
            pc = psA.tile([cout, SL], F32, tag="a")
            for j in range(3):  # taps (0,j)+(1,j) via Ta (+66 shift pair)
                nc.tensor.matmul(pc[:], wp[:, j, 0:cout], Ta[:, r0:r0 + 8, j:j + W],
                                 start=(j == 0), stop=False)
            nc.tensor.matmul(pc[:], wp[:, 3, 0:cout], Tb[:, r0 + 2:r0 + 10, 0:W],
                             start=False, stop=False)   # taps (2,0)+(2,1)
            nc.tensor.matmul(pc[:], wp[:, 4, 0:cout], Tb[:, r0 + 2:r0 + 10, 2:2 + W],
                             start=False, stop=True)    # tap (2,2), upper w = 0
            return pc

        # ========== conv1 (CBR) + transposes + CAM1 energy (interleaved) ==
        camE = psT.tile([C, C], F32, tag="tr")

        def cam_chunk_mms(dst, ch_base, src):
            for j in range(4):
                ch = ch_base + j
                nc.tensor.matmul(dst, src[:, ch, 0:C], src[:, ch, :],
                                 start=(ch == 0), stop=(ch == NCH - 1))

        for s in range(NSL):
            pc = conv_pair_slice(s, w1p, C, T66, T1)
            sl = slice(s * SL, (s + 1) * SL)
            # BN bias + relu  (weights pre-scaled on host)
            nc.vector.tensor_scalar(feat[:, sl], pc, b1v, 0.0,
                                    mybir.AluOpType.add, mybir.AluOpType.max)
            nc.scalar.copy(out=featB[:, sl], in_=feat[:, sl])
            for j in range(4):
                ch = s * 4 + j
                pool, tag = pe_pool(j)
                pt = pool.tile([128, C], F32, tag=tag)
                nc.tensor.transpose(pt, feat[:, ch * 128:(ch + 1) * 128],
                                    identity_f[0:C, 0:C])
                nc.vector.tensor_copy(out=xfT[:, ch, :], in_=pt)
            if s >= 1:
                cam_chunk_mms(camE, (s - 1) * 4, xfT)
        cam_chunk_mms(camE, (NSL - 1) * 4, xfT)

        attT1 = _cam_softmax(nc, misc, psT, camE, identity_b)

        q_rep = big.tile([128, N], BF16, tag="q_sb")
        S_all = big.tile([128, NCH, 128], BF16, tag="k_sb")
        valT = big.tile([128, NCH, 128], BF16, tag="valT")
        nc.gpsimd.memset(valT, 0.0)
        nc.gpsimd.memset(valT[:, :, C:C + 1], 1.0)

        def qkv_slice(s):
            sl = slice(s * SL, (s + 1) * SL)
            # q/k 1x1 convs with 16x-replicated output columns (+bias) -> bf16.
            # q lands directly in the 128-partition layout the K=128 energy
            # matmuls need; k gets masked into the block-diagonal stationaries.
            pq = psA.tile([128, SL], F32, tag="a")
            nc.tensor.matmul(pq, qkr[:, 0:128], y1b[:, sl], start=True, stop=True)
            nc.scalar.activation(out=q_rep[:, sl], in_=pq,
                                 func=mybir.ActivationFunctionType.Identity,
                                 bias=qkb[:, 0:1])
            pk = psA.tile([128, SL], F32, tag="a")
            nc.tensor.matmul(pk, qkr[:, 128:256], y1b[:, sl], start=True, stop=True)
            krepb = misc.tile([128, SL], BF16, tag="krep")
            nc.scalar.activation(out=krepb, in_=pk,
                                 func=mybir.ActivationFunctionType.Identity,
                                 bias=qkb[:, 1:2])
            for j in range(4):
                ch = s * 4 + j
                nc.vector.tensor_mul(S_all[:, ch, :], krepb[:, j * 128:(j + 1) * 128],
                                     maskb)
            # valT chunks (v 1x1 conv in transposed layout; vb folded in later)
            for j in range(4):
                ch = s * 4 + j
                pool, tag = pe_pool(j)
                pv = pool.tile([128, C], F32, tag=tag)
                nc.tensor.matmul(pv, y1b[:, ch * 128:(ch + 1) * 128], vwt[:],
                                 start=True, stop=True)
                nc.vector.tensor_copy(out=valT[:, ch, 0:C], in_=pv)

        # ========== CAM1 apply (bf16) -> y1 (+bf16 copy), then qk/valT =====
        for s in range(NSL):
            sl = slice(s * SL, (s + 1) * SL)
            pa = psA.tile([C, SL], F32, tag="a")
            nc.tensor.matmul(pa, attT1[:], featB[:, sl], start=True, stop=True)
            # y1 = g1*pa + feat ; y1b = bf16 copy
            nc.vector.scalar_tensor_tensor(
                out=y1[:, sl], in0=pa, scalar=g1v, in1=feat[:, sl],
                op0=mybir.AluOpType.mult, op1=mybir.AluOpType.add)
            nc.scalar.copy(out=y1b[0:C, sl], in_=y1[:, sl])
            if s >= 1:
                qkv_slice(s - 1)
        qkv_slice(NSL - 1)

        # y2 reuses feat's slot (feat is dead after the CAM1 apply loop);
        # y2b reuses y1b's slot (y1b dead after qkv_slice(7)).
        y2 = maps.tile([C, N], F32, tag="feat")
        y2b = maps.tile([128, N], BF16, tag="y1b")

        if phases >= 2:
            # ================= PAM (software-pipelined) =================
            def energy_slice(s, prev_apply):
                """Energy+exp for slice s, with slice s-1's apply matmuls
                interleaved between energy groups so the PE never stalls on
                the scalar engine's exp."""
                sl = slice(s * SL, (s + 1) * SL)
                expT = expp.tile([128, NCH, SL], BF16, tag="expT")
                po, prev_expT = None, None
                if prev_apply is not None:
                    po = psA.tile([128, SL], F32, tag="a")
                    prev_expT = prev_apply
                cursor = 0
                for gi, (c0, gw) in enumerate(E_GROUPS):
                    pool, tag = (peA, "eA") if gw == 3 else (peB, "eB")
                    ep = pool.tile([128, gw, SL], F32, tag=tag)
                    for j in range(gw):
                        ch = c0 + j
                        nc.tensor.matmul(ep[:, j, :], S_all[:, ch, :],
                                         q_rep[:, sl], start=True, stop=True)
                    nc.scalar.activation(out=expT[:, c0:c0 + gw, :], in_=ep,
                                         func=mybir.ActivationFunctionType.Exp)
                    if po is not None:
                        # spread the 32 apply matmuls over the 13 groups
                        target = (NCH * (gi + 1) + len(E_GROUPS) - 1) // len(E_GROUPS)
                        while cursor < min(target, NCH):
                            nc.tensor.matmul(po, valT[:, cursor, :],
                                             prev_expT[:, cursor, :],
                                             start=(cursor == 0),
                                             stop=(cursor == NCH - 1))
                            cursor += 1
                while po is not None and cursor < NCH:
                    nc.tensor.matmul(po, valT[:, cursor, :], prev_expT[:, cursor, :],
                                     start=(cursor == 0), stop=(cursor == NCH - 1))
                    cursor += 1
                return expT, po

            def accum_tail(po):
                # drain the finished apply accumulation to SBUF; recip here so
                # the later ones-broadcast matmul never waits.
                poS = misc.tile([C + 1, SL], F32, tag="poS")
                nc.vector.tensor_copy(out=poS, in_=po[0:C + 1, :])
                r1 = misc.tile([1, SL], F32, tag="r1")
                nc.vector.reciprocal(out=r1, in_=poS[C:C + 1, :])
                r1g = misc.tile([1, SL], F32, tag="r1g")
                nc.vector.tensor_scalar_mul(r1g, r1, gpv[0:1])  # fold gamma_p
                return poS, r1g

            def norm_slice(s, poS, r1g):
                sl = slice(s * SL, (s + 1) * SL)
                rb = misc.tile([C, SL], F32, tag="rb")
                nc.gpsimd.partition_broadcast(out_ap=rb, in_ap=r1g)
                t2 = misc.tile([C, SL], F32, tag="t2")
                nc.vector.tensor_mul(t2, poS[0:C, :], rb)
                # y2 = (t2 + gp*vb) + y1
                nc.vector.scalar_tensor_tensor(
                    out=y2[:, sl], in0=t2, scalar=gpvbv, in1=y1[:, sl],
                    op0=mybir.AluOpType.add, op1=mybir.AluOpType.add)
                nc.vector.tensor_copy(out=y2b[0:C, sl], in_=y2[:, sl])

            pos = [None] * NSL
            prev_expT = None
            for s in range(NSL):
                cur_expT, po_prev = energy_slice(s, prev_expT)
                if po_prev is not None:
                    pos[s - 1] = accum_tail(po_prev)
                if s >= 2:
                    norm_slice(s - 2, *pos[s - 2])
                prev_expT = cur_expT
            # final apply (slice 7) has no following energy slice
            po = psA.tile([128, SL], F32, tag="a")
            for ch in range(NCH):
                nc.tensor.matmul(po, valT[:, ch, :], prev_expT[:, ch, :],
                                 start=(ch == 0), stop=(ch == NCH - 1))
            pos[NSL - 1] = accum_tail(po)
            norm_slice(NSL - 2, *pos[NSL - 2])
            norm_slice(NSL - 1, *pos[NSL - 1])

        # padded conv2 input tiles, reusing conv1's T66/T1 slots
        y3T66 = pads.tile([128, HP + 1, WP], BF16, tag="t66")
        y3T1 = pads.tile([128, HP + 1, WP], BF16, tag="t1")
        nc.gpsimd.memset(y3T66, 0.0)
        nc.gpsimd.memset(y3T1, 0.0)

        if phases >= 3:
            # ================= CAM2 =================
            camE2 = psT.tile([C, C], F32, tag="tr")
            xfT2 = big.tile([128, NCH, C], F32, tag="xfT")  # reuse xfT slot
            for ch in range(NCH):
                pool, tag = pe_pool(ch)
                pt = pool.tile([128, C], F32, tag=tag)
                nc.tensor.transpose(pt, y2[:, ch * 128:(ch + 1) * 128],
                                    identity_f[0:C, 0:C])
                nc.scalar.copy(out=xfT2[:, ch, :], in_=pt)
                if ch >= 2:
                    cc = ch - 2
                    nc.tensor.matmul(camE2, xfT2[:, cc, 0:C], xfT2[:, cc, :],
                                     start=(cc == 0), stop=False)
            for cc in (NCH - 2, NCH - 1):
                nc.tensor.matmul(camE2, xfT2[:, cc, 0:C], xfT2[:, cc, :],
                                 start=False, stop=(cc == NCH - 1))
            attT2 = _cam_softmax(nc, misc, psT, camE2, identity_b)

            def conv2_slice(s):
                pc = conv_pair_slice(s, w2p, CI, y3T66, y3T1)
                nc.vector.tensor_scalar(out32[:, s * SL:(s + 1) * SL], pc, b2v, 0.0,
                                        mybir.AluOpType.add, mybir.AluOpType.max)

            out32 = maps.tile([CI, N], BF16, tag="y1")  # reuse y1 slot (dead soon)
            for s in range(NSL):
                r0 = s * 8
                sl = slice(s * SL, (s + 1) * SL)
                pa = psA.tile([C, SL], F32, tag="a")
                nc.tensor.matmul(pa, attT2[:], y2b[0:C, sl], start=True, stop=True)
                # y3 = g2*pa + y2  (strided into the padded conv2 input)
                nc.vector.scalar_tensor_tensor(
                    out=y3T66[0:C, 1 + r0:9 + r0, 1:W + 1],
                    in0=pa[:].rearrange("c (h w) -> c h w", h=8),
                    scalar=g2v,
                    in1=y2[:, sl].rearrange("c (h w) -> c h w", h=8),
                    op0=mybir.AluOpType.mult, op1=mybir.AluOpType.add)
                # shifted copies for conv2's tap pairs, per slice
                srcv = y3T66[0:C, 1 + r0:9 + r0, 1:W + 1]
                nc.sync.dma_start(out=y3T1[0:C, 1 + r0:9 + r0, 1:W + 1], in_=srcv)
                nc.scalar.dma_start(out=y3T66[C:128, r0:8 + r0, 1:W + 1], in_=srcv)
                nc.scalar.dma_start(out=y3T1[C:128, 1 + r0:9 + r0, 0:W], in_=srcv)
                if s >= 2:
                    conv2_slice(s - 2)

        if phases >= 4:
            # ====== conv2 (CBR) + qkv partials + split AllReduce ======
            cc_in = dram.tile([2, 3, N // 2], F32)
            cc_out = dram.tile([2, 3, N // 2], F32)
            dbg32 = None
            conv2_slice(NSL - 2)
            conv2_slice(NSL - 1)
            if phases == 4:
                dbg32 = misc.tile([CI, SL], BF16, tag="dbg32", bufs=1)
                nc.vector.tensor_copy(out=dbg32, in_=out32[:, 0:SL])
            # qkv partials batched after conv2 (tiny-K matmuls kept out of the
            # conv stream), halves AllReduced as soon as their partials exist
            for s in range(NSL):
                pf = psT.tile([3, SL], F32, tag="tr")
                nc.tensor.matmul(pf, sqkvt[:], out32[:, s * SL:(s + 1) * SL],
                                 start=True, stop=True)
                qkvp = misc.tile([3, SL], F32, tag="qkvp")
                nc.vector.tensor_copy(out=qkvp, in_=pf)
                half, off = s // 4, (s % 4) * SL
                nc.sync.dma_start(out=cc_in[half, :, off:off + SL], in_=qkvp)
                if phases >= 5 and s == 3:
                    nc.gpsimd.collective_compute(
                        "AllReduce", mybir.AluOpType.add,
                        replica_groups=REPLICA_GROUPS,
                        ins=[cc_in[0:1, :, :].opt()], outs=[cc_out[0:1, :, :].opt()])
            if phases >= 5:
                nc.gpsimd.collective_compute(
                    "AllReduce", mybir.AluOpType.add,
                    replica_groups=REPLICA_GROUPS,
                    ins=[cc_in[1:2, :, :].opt()], outs=[cc_out[1:2, :, :].opt()])

        if phases >= 5:
            # ============ tiny row-attention on the reduced q/k/v ============
            qS = misc.tile([H, W], F32, tag="qS")
            kS = misc.tile([H, W], F32, tag="kS")
            vS = misc.tile([H, W], F32, tag="vS")
            for hh in range(2):
                rows = slice(hh * 32, hh * 32 + 32)
                nc.sync.dma_start(out=qS[rows, :], in_=cc_out[hh:hh + 1, 0:1, :]
                                  .rearrange("a o (h w) -> (a o h) w", h=32))
                nc.scalar.dma_start(out=kS[rows, :], in_=cc_out[hh:hh + 1, 1:2, :]
                                    .rearrange("a o (h w) -> (a o h) w", h=32))
                nc.sync.dma_start(out=vS[rows, :], in_=cc_out[hh:hh + 1, 2:3, :]
                                  .rearrange("a o (h w) -> (a o h) w", h=32))
            qSb = misc.tile([H, W], BF16, tag="qSb")
            nc.vector.tensor_copy(out=qSb, in_=qS)
            kSb = misc.tile([H, W], BF16, tag="kSb")
            nc.vector.tensor_copy(out=kSb, in_=kS)
            vSb = misc.tile([H, W], BF16, tag="vSb")
            nc.vector.tensor_copy(out=vSb, in_=vS)
            pq = psT.tile([W, H], BF16, tag="tr")
            nc.tensor.transpose(pq, qSb[:], identity_b[0:H, 0:H])
            qT = misc.tile([W, H], BF16, tag="qT")
            nc.vector.tensor_copy(out=qT, in_=pq)
            pk = psT.tile([W, H], BF16, tag="tr")
            nc.tensor.transpose(pk, kSb[:], identity_b[0:H, 0:H])
            kT = misc.tile([W, H], BF16, tag="kT")
            nc.vector.tensor_copy(out=kT, in_=pk)

            pE = psT.tile([H, H], F32, tag="tr")
            nc.tensor.matmul(pE, qT[:], kT[:], start=True, stop=True)
            m2 = misc.tile([H, 1], F32, tag="fm2")
            nc.vector.reduce_max(out=m2, in_=pE, axis=mybir.AxisListType.X, negate=True)
            exf = misc.tile([H, H], F32, tag="fex")
            sf = misc.tile([H, 1], F32, tag="fs")
            nc.scalar.activation(out=exf, in_=pE, func=mybir.ActivationFunctionType.Exp,
                                 bias=m2, scale=1.0, accum_out=sf)
            rf = misc.tile([H, 1], F32, tag="frf")
            nc.vector.reciprocal(out=rf, in_=sf)
            alpha = misc.tile([H, H], BF16, tag="falpha")
            nc.vector.tensor_scalar_mul(alpha, exf, rf)
            pAT = psT.tile([H, H], BF16, tag="tr")
            nc.tensor.transpose(pAT, alpha[:], identity_b[0:H, 0:H])
            alphaT = misc.tile([H, H], BF16, tag="falphaT")
            nc.vector.tensor_copy(out=alphaT, in_=pAT)
            pO = psT.tile([H, W], F32, tag="tr")
            nc.tensor.matmul(pO, alphaT[:], vSb[:], start=True, stop=True)
            res = misc.tile([H, W], F32, tag="fres")
            nc.vector.tensor_add(res, pO, vS)
            nc.sync.dma_start(out=out_ext[:], in_=res)

        if phases == 1:
            nc.sync.dma_start(out=out_ext[:], in_=y1[:, 0:W])
        elif phases in (2, 3):
            nc.sync.dma_start(out=out_ext[:], in_=y2[:, 0:W])
        elif phases == 4:
            nc.gpsimd.dma_start(out=out_ext[0:32, :], in_=dbg32[0:32, 0:W])

    nc.compile()
    return nc


_NC_CACHE = {}


def get_nc():
    if "nc" not in _NC_CACHE:
        _NC_CACHE["nc"] = build_nc()
    return _NC_CACHE["nc"]


def _fold_bn(w, s, b, m, v):
    a = s / np.sqrt(v + EPS)
    return w * a[:, None, None, None], b - m * a


def make_in_maps(inputs):
    inp = {k: np.asarray(v, np.float32) for k, v in inputs.items()}
    x = inp["x"]

    def conv_pack(wname):
        w, bb = _fold_bn(inp[wname + "_w"], inp[wname + "_s"], inp[wname + "_b"],
                         inp[wname + "_m"], inp[wname + "_v"])
        # per-tap lhsT [ci, co]; tap-pair stacks [128, 5, co]
        wt = w.transpose(2, 3, 1, 0)  # [dy, dx, ci, co]
        co = wt.shape[-1]
        packs = np.zeros((128, 5, co), np.float32)
        for j in range(3):           # (0,j) + (1,j) via T66
            packs[0:C, j] = wt[0, j]
            packs[C:128, j] = wt[1, j]
        packs[0:C, 3] = wt[2, 0]     # (2,0) + (2,1) via T1
        packs[C:128, 3] = wt[2, 1]
        packs[0:C, 4] = wt[2, 2]     # (2,2) single; upper half zero
        return packs, bb

    w1p_a, b1_a = conv_pack("c5c")   # branch A first conv
    w1p_b, b1_b = conv_pack("c5a")   # branch B first conv
    w2p_a, b2_a = conv_pack("c51")
    w2p_b, b2_b = conv_pack("c52")

    qw = inp["pam_qw"][:, :, 0, 0].T   # [C, 8]
    kw = inp["pam_kw"][:, :, 0, 0].T   # [C, 8]
    qkr = np.zeros((128, 256), np.float32)
    qkb2 = np.zeros((128, 2), np.float32)
    for j in range(16):
        qkr[0:C, 8 * j:8 * j + 8] = qw
        qkr[0:C, 128 + 8 * j:128 + 8 * j + 8] = kw
        qkb2[8 * j:8 * j + 8, 0] = inp["pam_qb"]
        qkb2[8 * j:8 * j + 8, 1] = inp["pam_kb"]
    mask = np.zeros((128, 128), np.float32)
    for j in range(16):
        mask[8 * j:8 * j + 8, 8 * j:8 * j + 8] = 1.0
    vwt = np.zeros((128, C), np.float32)
    vwt[0:C] = inp["pam_vw"][:, :, 0, 0].T
    vb = inp["pam_vb"]
    gp = float(inp["pam_g"][0])
    gc = float(inp["cam_g"][0])

    sq = inp["sq_w"][0, :, 0, 0]
    sk = inp["sk_w"][0, :, 0, 0]
    sv = inp["sv_w"][0, :, 0, 0]

    # pre-padded x [B, C, 66, 66] bf16
    xpad = np.zeros((B, C, HP, WP), np.float32)
    xpad[:, :, 1:H + 1, 1:W + 1] = x.reshape(B, C, H, W)
    xpad = xpad.reshape(B, C, PADN)

    in_maps = []
    for b in range(B):
        for br in range(2):  # 0 = branch A (CAM->PAM), 1 = branch B (PAM->CAM)
            is_a = (br == 0)
            vecs = np.zeros((C, 8), np.float32)
            vecs[:, 0] = b1_a if is_a else b1_b
            vecs[:, 1] = gc if is_a else 0.0
            vecs[:, 2] = 0.0 if is_a else gc
            vecs[:, 3] = gp
            vecs[:, 4] = gp * vb
            vecs[:CI, 5] = b2_a if is_a else b2_b
            half = slice(0, CI) if is_a else slice(CI, C)
            sqkvt = np.stack([sq[half], sk[half], sv[half]], axis=1)  # [32, 3]
            w1p = w1p_a if is_a else w1p_b
            w2p = w2p_a if is_a else w2p_b
            in_maps.append({
                "x": np.ascontiguousarray(xpad[b]).astype(ml_dtypes.bfloat16),
                "w1p": np.ascontiguousarray(w1p.reshape(128, 5 * C)).astype(ml_dtypes.bfloat16),
                "w2p": np.ascontiguousarray(w2p.reshape(128, 5 * CI)).astype(ml_dtypes.bfloat16),
                "qkr": np.ascontiguousarray(qkr).astype(ml_dtypes.bfloat16),
                "qkb": qkb2,
                "mask": np.ascontiguousarray(mask).astype(ml_dtypes.bfloat16),
                "vwt": np.ascontiguousarray(vwt).astype(ml_dtypes.bfloat16),
                "sqkvt": np.ascontiguousarray(sqkvt).astype(ml_dtypes.bfloat16),
                "vecs": vecs,
            })
    return in_maps


def kernel(_res_cache={}, **inputs):
    nc = get_nc()
    in_maps = make_in_maps(inputs)
    res = run_bass_kernel_spmd(nc, in_maps, list(range(8)))
    _res_cache["last"] = res
    out = np.stack([res.results[2 * b]["out"] for b in range(B)])
    return out[:, None].astype(np.float32)
